# revision 33
# baseline (speedup 1.0000x reference)
"""GAT (3-layer, DGL-style) forward pass on 8 Trainium2 NeuronCores.

Strategy (dst-node sharded, graph-parallel):
  - Nodes are partitioned into 8 contiguous shards (dst ownership); edges are
    grouped by owner(dst), then by 128-node dst tile, then by table quarter
    (int16 gather-index range).  Quarters coincide with AllGather chunks so
    each quarter's gathers depend only on one chunked collective.
  - Per layer, each core computes feat_aug = h_shard @ [W | W@Al | W@Ar]; the
    [feat | el] part is AllGathered (in 4 chunks, overlapped with the previous
    edge phase) into a compact row table, then re-strided to a 512B-pitch
    table so dma_gather (elem 512B) delivers feat AND el per edge.  er stays
    local (only own-dst er is ever needed).
  - Edge phase per dst tile: src rows are fetched with dma_gather (4 SWDGE
    queues, idx streams padded to full 128-chunks with index 0 so no memsets
    are needed), er is expanded edge-wise with a host-precomputed transposed
    one-hot matmul, ex = exp(leaky_relu(el+er)) runs on the scalar engine,
    and one-hot matmuls accumulate sum(ex*feat) and sum(ex) per dst node in
    PSUM (softmax max-subtraction cancels; logits are O(1)).
  - Epilogue: normalize, residual, bias, ELU (scalar-engine heavy), plus the
    next layer's dense matmul fused in via an on-chip transpose (layer 3:
    projected residual and the final classifier are fused the same way).

All core-dependent information lives in per-core input tensors, so every core
runs an identical SPMD program.
"""

import sys

import numpy as np

for _p in ("/opt/trn_rl_repo",):
    if _p not in sys.path:
        sys.path.insert(0, _p)

import ml_dtypes

BF16 = ml_dtypes.bfloat16

P = 128
NEG_SLOPE = 0.2
HID = 32
N_CLS = 40
N_CORES = 8
NQUARTER = 4
NI_HW_MAX = 1024  # dma_gather num_idxs beyond this crashes (HW-probed)
AG_TILES = (25, 25, 25, 23)  # tiles per AllGather chunk (sum = T = 98)

_PROGRAM_CACHE = {}


# ----------------------------------------------------------------------------
# Host-side preparation (index manipulation / sharding only)
# ----------------------------------------------------------------------------

def _make_waug(W, al, ar, with_al):
    """[in, H*D] -> [in, H*D (+H if with_al) +H] f32: [W | (W@Al) | W@Ar]."""
    H, D = al.shape
    W64 = W.astype(np.float64)

    def proj(a):
        A = np.zeros((H * D, H), np.float64)
        A[np.arange(H * D), np.arange(H * D) // D] = a.reshape(-1).astype(np.float64)
        return W64 @ A

    parts = [W64]
    if with_al:
        parts.append(proj(al))
    parts.append(proj(ar))
    return np.concatenate(parts, axis=1).astype(np.float32)


def _prepare(x, src, dst, n_cores=N_CORES):
    n_nodes = x.shape[0]
    assert n_nodes % n_cores == 0
    shard = n_nodes // n_cores
    shard_pad = ((shard + P - 1) // P) * P
    T = shard_pad // P
    assert sum(AG_TILES) == T
    rows_k = np.array([t * P for t in AG_TILES])            # per-core rows/chunk
    rbase = np.concatenate([[0], np.cumsum(rows_k)])[:NQUARTER]
    qrows = rows_k * n_cores                                 # table rows/quarter
    assert qrows.max() <= 32767, "quarter must fit int16 index range"

    # table row (within quarter k) of src node s: owner*rows_k[k] + local off
    owner_s = src // shard
    lr = src - owner_s * shard
    quarter = np.minimum(lr // rows_k[0], NQUARTER - 1)
    qidx = (owner_s * rows_k[quarter] + (lr - rbase[quarter])).astype(np.int16)

    owner = dst // shard
    local = dst - owner * shard
    tloc = local // P
    doff = (local % P).astype(np.int16)

    group = (owner * T + tloc) * NQUARTER + quarter  # [E]
    n_groups = n_cores * T * NQUARTER
    counts = np.bincount(group, minlength=n_groups).reshape(n_cores, T, NQUARTER)
    ni_tq = counts.max(axis=0)  # [T, NQ] shared across cores (SPMD)
    assert ni_tq.max() <= NI_HW_MAX, ni_tq.max()
    ch_tq = (ni_tq + P - 1) // P  # chunks per (tile, quarter)
    nch_t = ch_tq.sum(axis=1)  # [T]
    NCHMAX = int(nch_t.max())
    b0_tq = np.concatenate(
        [np.zeros((T, 1), np.int64), np.cumsum(ch_tq, axis=1)[:, :3]], axis=1
    )
    ICW = NCHMAX * 8  # idx columns (16 idxs per column)

    # order edges by (core, tile, quarter); position within group
    order = np.argsort(group, kind="stable")
    g_sorted = group[order]
    starts = np.zeros(n_groups + 1, np.int64)
    np.cumsum(np.bincount(group, minlength=n_groups), out=starts[1:])
    pos = np.arange(len(order)) - starts[g_sorted]

    # raw num_idxs means tail slots beyond each quarter's count get no
    # descriptors; their SBUF slots keep stale-but-finite data from a
    # previous tile (one-hot zeros mask them out of the aggregation)
    gidx_all = np.zeros((n_cores, T, NCHMAX * P), np.int16)
    dst_pb = np.full((n_cores, T, P, NCHMAX), -1, np.int16)

    oc = g_sorted // (T * NQUARTER)
    tc = (g_sorted // NQUARTER) % T
    qc = g_sorted % NQUARTER
    # stream position within the tile's idx stream == slot index
    slot = b0_tq[tc, qc] * P + pos
    gidx_all[oc, tc, slot] = qidx[order]
    dst_pb[oc, tc, slot % P, slot // P] = doff[order]

    # wrap gather indices: stream position i at [i%16, i//16], replicated x8
    gidx_w = gidx_all.reshape(n_cores, T, ICW, 16).transpose(0, 1, 3, 2)
    gidx_w = np.broadcast_to(gidx_w[:, :, None, :, :],
                             (n_cores, T, 8, 16, ICW))
    gidx_w = np.ascontiguousarray(gidx_w).reshape(n_cores, T * P, ICW)

    # one-hots ship as fp8 (0/1 exact): transposed for er expansion
    # ohT[t, i, b*128+p] = (dst_pb==i), untransposed for aggregation
    FP8 = ml_dtypes.float8_e4m3fn
    i_ar = np.arange(P, dtype=np.int16)
    oht = (dst_pb[:, :, None, :, :] == i_ar[None, None, :, None, None])
    oht = oht.transpose(0, 1, 2, 4, 3).astype(FP8)  # [C, T, i, b, p]
    oht = np.ascontiguousarray(oht.reshape(n_cores, T * P, NCHMAX * P))
    ohu = (dst_pb[:, :, :, :, None] == i_ar[None, None, None, None, :])
    ohu = np.ascontiguousarray(
        ohu.astype(FP8).reshape(n_cores, T * P, NCHMAX * P))

    xT_per_core = []
    for c in range(n_cores):
        xs = x[c * shard:(c + 1) * shard].astype(np.float32)
        if shard_pad != shard:
            xs = np.concatenate(
                [xs, np.zeros((shard_pad - shard, xs.shape[1]), np.float32)], 0)
        xT_per_core.append(np.ascontiguousarray(xs.T))

    return dict(
        shard=shard, shard_pad=shard_pad, T=T,
        rows_k=rows_k.tolist(), rbase=rbase.tolist(), qrows=qrows.tolist(),
        NCHMAX=NCHMAX, ICW=ICW,
        ni_tq=ni_tq.tolist(), ch_tq=ch_tq.tolist(), nch_t=nch_t.tolist(),
        b0_tq=b0_tq.tolist(),
        gidx_per_core=[np.ascontiguousarray(gidx_w[c]) for c in range(n_cores)],
        oht_per_core=[np.ascontiguousarray(oht[c]) for c in range(n_cores)],
        ohu_per_core=[np.ascontiguousarray(ohu[c]) for c in range(n_cores)],
        xT_per_core=xT_per_core,
    )


# ----------------------------------------------------------------------------
# Device program
# ----------------------------------------------------------------------------

def _build_program(n_cores, plan, has_bias):
    from concourse import bacc, bass, tile
    import concourse.mybir as mybir
    from concourse.masks import make_identity

    dt = mybir.dt
    f32, bf16, i16 = dt.float32, dt.bfloat16, dt.int16
    f8 = dt.float8e4
    Alu = mybir.AluOpType
    Act = mybir.ActivationFunctionType

    shard, SP, T = plan["shard"], plan["shard_pad"], plan["T"]
    rows_k, rbase = plan["rows_k"], plan["rbase"]
    NCHMAX, ICW = plan["NCHMAX"], plan["ICW"]
    ni_tq, ch_tq = plan["ni_tq"], plan["ch_tq"]
    nch_t, b0_tq = plan["nch_t"], plan["b0_tq"]
    rg = [list(range(n_cores))]
    # tile index at which each AG chunk's input rows are complete
    chunk_end_t = np.cumsum(AG_TILES) - 1  # [24, 49, 74?, 97] -> 24,49,74,97

    nc = bacc.Bacc("TRN2", target_bir_lowering=False, debug=False,
                   num_devices=n_cores, num_swdge_queues=4)

    xT = nc.dram_tensor("xT", [P, SP], f32, kind="ExternalInput")
    gidx = nc.dram_tensor("gidx", [T * P, ICW], i16, kind="ExternalInput")
    ohtd = nc.dram_tensor("ohtd", [T * P, NCHMAX * P], f8, kind="ExternalInput")
    ohud = nc.dram_tensor("ohud", [T * P, NCHMAX * P], f8, kind="ExternalInput")
    waug1 = nc.dram_tensor("waug1", [P, 136], f32, kind="ExternalInput")
    waug2 = nc.dram_tensor("waug2", [P, 136], f32, kind="ExternalInput")
    waug3 = nc.dram_tensor("waug3", [P, P], f32, kind="ExternalInput")
    res3w = nc.dram_tensor("res3w", [P, HID], f32, kind="ExternalInput")
    wfc = nc.dram_tensor("wfc", [HID, N_CLS], f32, kind="ExternalInput")
    bias_d = [None] * 4
    bias_shapes = [(P, P), (P, P), (P, HID), (P, N_CLS)]
    for i, hb in enumerate(has_bias):
        if hb:
            bias_d[i] = nc.dram_tensor(f"bias{i}", list(bias_shapes[i]), f32,
                                       kind="ExternalInput")
    out_e = nc.dram_tensor("out", [shard, N_CLS], f32, kind="ExternalOutput")

    # AllGather staging: layers 1-2 use 512B-pitch rows [feat(128)|el(4)|pad]
    # gathered directly (elem 512B); layer 3 a compact 256B row.
    agin = [nc.dram_tensor(f"agin{l}", [SP, 256 if l < 2 else P], bf16,
                           kind="Internal") for l in range(3)]
    tablec = [None, None,
              [nc.dram_tensor(f"tablec2_{k}", [n_cores * rows_k[k], P],
                              bf16, kind="Internal", addr_space="Shared")
               for k in range(NQUARTER)]]
    tablep = [[nc.dram_tensor(f"tablep{l}_{k}", [n_cores * rows_k[k], 256],
                              bf16, kind="Internal", addr_space="Shared")
               for k in range(NQUARTER)] for l in range(2)]
    h1d = nc.dram_tensor("h1d", [SP, P], f32, kind="Internal")

    with tile.TileContext(nc) as tc:
        with (
            tc.tile_pool(name="const", bufs=1) as cpool,
            tc.tile_pool(name="big", bufs=1) as bigpool,
            tc.tile_pool(name="gth", bufs=8) as gpool,
            tc.tile_pool(name="oht", bufs=8) as opool,
            tc.tile_pool(name="work", bufs=6) as wpool,
            tc.tile_pool(name="wsm", bufs=8) as spool,
            tc.tile_pool(name="pagg", bufs=2, space="PSUM") as p_agg,
            tc.tile_pool(name="ptr", bufs=2, space="PSUM") as p_tr,
            tc.tile_pool(name="pdn", bufs=2, space="PSUM") as p_dn,
            tc.tile_pool(name="per", bufs=1, space="PSUM") as p_er,
            tc.tile_pool(name="prs", bufs=1, space="PSUM") as p_rs,
        ):
            ident = cpool.tile([P, P], f32)
            make_identity(nc, ident[:])

            w1_sb = cpool.tile([P, 136], f32)
            nc.sync.dma_start(w1_sb[:], waug1[:, :])
            w2_sb = cpool.tile([P, 136], f32)
            nc.sync.dma_start(w2_sb[:], waug2[:, :])
            w3_sb = cpool.tile([P, P], f32)
            nc.sync.dma_start(w3_sb[:], waug3[:, :])
            res3_sb = cpool.tile([P, HID], f32)
            nc.sync.dma_start(res3_sb[:], res3w[:, :])
            wfc_sb = cpool.tile([HID, N_CLS], f32)
            nc.sync.dma_start(wfc_sb[:], wfc[:, :])
            bias_sb = [None] * 4
            for i, d in enumerate(bias_d):
                if d is not None:
                    bias_sb[i] = cpool.tile(list(bias_shapes[i]), f32)
                    nc.sync.dma_start(bias_sb[i][:], d[:, :])

            h2T = bigpool.tile([P, SP], f32)          # for layer-3 residual
            er_sb = [bigpool.tile([P, T, 4], bf16, name=f"er{i}_sb") for i in range(3)]

            def bcast_mid(ap, n):
                return bass.AP(ap.tensor, ap.offset, [ap.ap[0], [0, n], ap.ap[1]])

            def fire_chunk(layer_i, k):
                """AllGather chunk k of layer layer_i's table."""
                r0, r1 = rbase[k], rbase[k] + rows_k[k]
                out_t = tablep[layer_i][k] if layer_i < 2 else tablec[2][k]
                nc.gpsimd.collective_compute(
                    "AllGather", Alu.bypass, replica_groups=rg,
                    ins=[agin[layer_i][r0:r1, :]],
                    outs=[out_t[:, :]])

            def dense_tile(t, lhsT_ap, w_sb, ncols, layer_i):
                """feat_aug for tile t of next layer: write agin + er_sb."""
                ps = p_dn.tile([P, ncols], f32, tag="ps_dense")
                nc.tensor.matmul(ps[:], lhsT=lhsT_ap, rhs=w_sb[:], start=True,
                                 stop=True)
                acols = 132 if layer_i < 2 else P
                fsb = wpool.tile([P, acols], bf16, tag="fsb")
                nc.scalar.activation(fsb[:], ps[:, :acols], Act.Copy)
                nc.sync.dma_start(agin[layer_i][t * P:(t + 1) * P, :acols],
                                  fsb[:])
                if layer_i < 2:
                    nc.scalar.activation(er_sb[layer_i][:, t, :], ps[:, 132:136],
                                         Act.Copy)
                else:
                    nc.scalar.activation(er_sb[2][:, t, 0:1], ps[:, 33:34],
                                         Act.Copy)

            # layer-1 dense from xT, with chunked AllGather fired as soon as
            # each chunk's rows are written
            kq = 0
            for t in range(T):
                lh = wpool.tile([P, P], f32, tag="xt_t")
                nc.sync.dma_start(lh[:], xT[:, t * P:(t + 1) * P])
                dense_tile(t, lh[:], w1_sb, 136, 0)
                if kq < NQUARTER and t == chunk_end_t[kq]:
                    fire_chunk(0, kq)
                    kq += 1

            def edge_phase(layer):  # 1-based
                li = layer - 1
                H = 4 if layer < 3 else 1
                FE = H * HID
                act = layer < 3
                ES = 256 if act else P  # gather elem (bf16 elements)
                kq = 0
                for t in range(T):
                    r0 = t * P
                    NCH = nch_t[t]
                    gix = spool.tile([P, ICW], i16, tag="gix")
                    nc.scalar.dma_start(gix[:], gidx[r0:r0 + P, :])
                    oht_sb = opool.tile([P, NCHMAX, P], f8, tag="oht")
                    nc.scalar.dma_start(oht_sb[:, :NCH, :],
                                        ohtd[r0:r0 + P, :NCH * P])
                    oh = opool.tile([P, NCHMAX, P], f8, tag="ohu")
                    nc.sync.dma_start(oh[:, :NCH, :],
                                      ohud[r0:r0 + P, :NCH * P])
                    gsb = gpool.tile([P, NCHMAX, ES], bf16, tag="gsb")
                    if t < 8 and layer in (1, 3):
                        # first use of each rotating buffer (per shape): clear
                        # once so trimmed-pad slots never hold non-finite bits
                        nc.vector.memset(gsb[:], 0.0)
                    for q in range(NQUARTER):
                        niq = ni_tq[t][q]
                        if niq == 0:
                            continue
                        chq, b0 = ch_tq[t][q], b0_tq[t][q]
                        table_q = (tablep[li][q] if act else tablec[2][q])
                        nc.gpsimd.dma_gather(
                            gsb[:, b0:b0 + chq, :],
                            table_q[:, :],
                            gix[:, 8 * b0:8 * (b0 + chq)],
                            num_idxs=niq, num_idxs_reg=niq, elem_size=ES,
                            queue_num=q, single_packet=False,
                        )
                    # el comes straight out of the gathered row
                    el_ap = (gsb[:, :NCH, 128:132] if act
                             else gsb[:, :NCH, 32:33])
                    # er expansion via host transposed one-hot
                    pse = p_er.tile([P, NCHMAX * H], f32, tag="ps_er")
                    for c in range(NCH):
                        nc.tensor.matmul(
                            pse[:, c * H:(c + 1) * H],
                            lhsT=oht_sb[:, c, :], rhs=er_sb[li][:, t, :H],
                            start=True, stop=True)
                    esb = spool.tile([P, NCHMAX, H], f32, tag="e")
                    nc.vector.tensor_tensor(
                        out=esb[:, :NCH, :], in0=el_ap,
                        in1=pse[:, :NCH * H].rearrange("p (c h) -> p c h", h=H),
                        op=Alu.add)
                    # ex = exp(lrelu(e)) = max(exp(e), exp(0.2*e)): two scalar
                    # Exps (scale folds the slope in), one vector max written
                    # directly into g's trailing columns
                    e1 = spool.tile([P, NCHMAX, H], f32, tag="e1")
                    nc.scalar.activation(e1[:, :NCH, :], esb[:, :NCH, :],
                                         Act.Exp)
                    e2 = spool.tile([P, NCHMAX, H], f32, tag="e2")
                    nc.scalar.activation(e2[:, :NCH, :], esb[:, :NCH, :],
                                         Act.Exp, scale=NEG_SLOPE)
                    g = wpool.tile([P, NCHMAX, FE + H], bf16, tag="g")
                    nc.vector.tensor_tensor(out=g[:, :NCH, FE:FE + H],
                                            in0=e1[:, :NCH, :],
                                            in1=e2[:, :NCH, :], op=Alu.max)
                    # g = [feat*ex | ex]
                    nc.vector.tensor_tensor(
                        out=g[:, :NCH, 0:FE].rearrange("p c (h d) -> p c h d", h=H),
                        in0=gsb[:, :NCH, 0:FE].rearrange("p c (h d) -> p c h d", h=H),
                        in1=g[:, :NCH, FE:FE + H].to_broadcast([P, NCH, H, HID]),
                        op=Alu.mult)
                    # aggregate
                    psa = p_agg.tile([P, FE + H], f32, tag="ps_agg")
                    for c in range(NCH):
                        nc.tensor.matmul(psa[:], lhsT=oh[:, c, :], rhs=g[:, c, :],
                                         start=(c == 0), stop=(c == NCH - 1))
                    # epilogue
                    ssb = spool.tile([P, H], f32, tag="s")
                    nc.vector.tensor_scalar_max(ssb[:], psa[:, FE:FE + H], 1e-30)
                    rec = spool.tile([P, H], f32, tag="rec")
                    nc.vector.reciprocal(rec[:], ssb[:])
                    osb = wpool.tile([P, FE], f32, tag="osb")
                    nc.vector.tensor_tensor(
                        out=osb[:].rearrange("p (h d) -> p h d", h=H),
                        in0=psa[:, 0:FE].rearrange("p (h d) -> p h d", h=H),
                        in1=rec[:].to_broadcast([P, H, HID]), op=Alu.mult)
                    if layer == 2:
                        rsb = wpool.tile([P, P], f32, tag="rsb")
                        nc.sync.dma_start(rsb[:], h1d[r0:r0 + P, :])
                        nc.vector.tensor_tensor(out=osb[:], in0=osb[:],
                                                in1=rsb[:], op=Alu.add)
                    elif layer == 3:
                        psr = p_rs.tile([P, HID], f32, tag="ps_res")
                        nc.tensor.matmul(psr[:], lhsT=h2T[:, r0:r0 + P],
                                         rhs=res3_sb[:], start=True, stop=True)
                        nc.vector.tensor_tensor(out=osb[:], in0=osb[:],
                                                in1=psr[:], op=Alu.add)
                    if bias_sb[li] is not None:
                        nc.vector.tensor_tensor(out=osb[:], in0=osb[:],
                                                in1=bias_sb[li][:, :FE],
                                                op=Alu.add)
                    if act:  # ELU = relu(x) + min(exp(x),1) - 1
                        eo = wpool.tile([P, FE], f32, tag="eo")
                        nc.scalar.activation(eo[:], osb[:], Act.Exp)
                        nc.vector.tensor_scalar(out=eo[:], in0=eo[:],
                                                scalar1=1.0, scalar2=-1.0,
                                                op0=Alu.min, op1=Alu.add)
                        nc.vector.tensor_scalar_max(osb[:], osb[:], 0.0)
                        nc.vector.tensor_tensor(out=osb[:], in0=osb[:],
                                                in1=eo[:], op=Alu.add)
                    if layer == 1:
                        nc.sync.dma_start(h1d[r0:r0 + P, :], osb[:])
                    # transpose; feeds next dense / h2T / classifier
                    pst = p_tr.tile([P, P], f32, tag="ps_t")
                    nc.tensor.transpose(pst[:FE, :], osb[:], ident[:])
                    if layer == 1:
                        hts = wpool.tile([P, P], f32, tag="h_t")
                        nc.scalar.activation(hts[:], pst[:], Act.Copy)
                        dense_tile(t, hts[:], w2_sb, 136, 1)
                    elif layer == 2:
                        nc.scalar.activation(h2T[:, r0:r0 + P], pst[:], Act.Copy)
                        dense_tile(t, h2T[:, r0:r0 + P], w3_sb, P, 2)
                    else:
                        hts = spool.tile([HID, P], f32, tag="h3t")
                        nc.scalar.activation(hts[:], pst[:HID, :], Act.Copy)
                        psf = p_dn.tile([P, N_CLS], f32, tag="ps_dense")
                        nc.tensor.matmul(psf[:], lhsT=hts[:], rhs=wfc_sb[:],
                                         start=True, stop=True)
                        ofc = spool.tile([P, N_CLS], f32, tag="ofc")
                        nc.scalar.activation(ofc[:], psf[:], Act.Copy)
                        if bias_sb[3] is not None:
                            nc.vector.tensor_tensor(out=ofc[:], in0=ofc[:],
                                                    in1=bias_sb[3][:, :],
                                                    op=Alu.add)
                        rows = min(shard - r0, P)
                        if rows > 0:
                            nc.sync.dma_start(out_e[r0:r0 + rows, :],
                                              ofc[:rows, :])
                    # fire the next layer's table chunk as soon as its input
                    # rows (written by dense_tile above) are complete
                    if layer < 3 and kq < NQUARTER and t == chunk_end_t[kq]:
                        fire_chunk(layer, kq)
                        kq += 1

            edge_phase(1)
            edge_phase(2)
            edge_phase(3)

    nc.compile()
    return nc


def _get_program(n_cores, plan, has_bias):
    key = (n_cores, plan["shard"], plan["NCHMAX"], plan["ICW"],
           tuple(plan["nch_t"]), tuple(map(tuple, plan["ni_tq"])), has_bias)
    if key not in _PROGRAM_CACHE:
        _PROGRAM_CACHE[key] = _build_program(n_cores, plan, has_bias)
    return _PROGRAM_CACHE[key]


def _make_in_maps(prep, inputs, has_bias, n_cores=N_CORES):
    waug1 = _make_waug(inputs["W1"], inputs["al1"], inputs["ar1"], True)
    waug2 = _make_waug(inputs["W2"], inputs["al2"], inputs["ar2"], True)
    waug3 = _make_waug(inputs["W3"], inputs["al3"], inputs["ar3"], True)
    waug3 = np.concatenate(
        [waug3, np.zeros((P, P - waug3.shape[1]), np.float32)], axis=1)

    biases = []
    shapes = [(P, P), (P, P), (P, HID), (P, N_CLS)]
    for i, nm in enumerate(("b1", "b2", "b3", "bfc")):
        b = np.asarray(inputs[nm], np.float32).reshape(1, -1)
        biases.append(np.ascontiguousarray(np.broadcast_to(b, shapes[i])))
    in_maps = []
    for c in range(n_cores):
        m = dict(
            xT=prep["xT_per_core"][c],
            gidx=prep["gidx_per_core"][c],
            ohtd=prep["oht_per_core"][c],
            ohud=prep["ohu_per_core"][c],
            waug1=waug1, waug2=waug2, waug3=waug3,
            res3w=np.asarray(inputs["res3"], np.float32),
            wfc=np.asarray(inputs["Wfc"], np.float32),
        )
        for i, hb in enumerate(has_bias):
            if hb:
                m[f"bias{i}"] = biases[i]
        in_maps.append(m)
    return in_maps


def run_gat(inputs, n_cores=N_CORES, trace=False):
    """Builds (cached), runs on hardware, returns (output, BassKernelResults)."""
    from concourse import bass_utils

    x, src, dst = inputs["x"], inputs["src"], inputs["dst"]
    prep = _prepare(x, src, dst, n_cores)
    has_bias = tuple(
        bool(np.any(np.asarray(inputs[nm]))) for nm in ("b1", "b2", "b3", "bfc"))
    nc = _get_program(n_cores, prep, has_bias)
    in_maps = _make_in_maps(prep, inputs, has_bias, n_cores)
    res = bass_utils.run_bass_kernel_spmd(
        nc, in_maps, core_ids=list(range(n_cores)), trace=trace)
    out = np.concatenate([r["out"] for r in res.results], axis=0)
    return out[: x.shape[0]].astype(np.float32), res


def kernel(**inputs):
    out, _ = run_gat(inputs)
    return out
